# revision 11
# baseline (speedup 1.0000x reference)
"""Distributed Trainium2 kernel for a single causal attention head.

Module: k,q,v = x@W{k,q,v}.T ; a = softmax(causal(q@k.T/sqrt(64))) ; out = a@v
Shapes: x (4, 4096, 1024) f32; W* (64, 1024) f32; out (4, 4096, 64) f32.

Sharding (one SPMD launch, 8 cores, no collectives): 4 batches x 2
key-parity halves. Core c: batch b=c//2, parity p=c%2. The 32 key chunks
(128 tokens) of a batch are split by parity (even chunks -> p=0, odd ->
p=1): for query chunk j (512 tokens), each core processes exactly 2j+2
of its local key chunks; its two diagonal mask tiles arrive as input
data. The host hands each core x[b].T with token columns permuted so the
core's own-parity key blocks sit at even 128-block positions.

Row-tiled S matmuls: the S contraction is only 64-deep (head dim), so
each local key-chunk PAIR runs as TWO concurrent matmuls on disjoint
64-row halves of the PE array (tile_position row strips) -- the pair
costs ~512 cycles instead of 1024. To feed this:
  - Q projection is COL-tiled: two concurrent M=64 matmuls (stationary
    Wq, movings = the region's two 512-token halves) write partitions
    0-63 / 64-127 of one PSUM bank, halving Q-projection time. The [Q;Q]
    row duplication that row-strip-1 S matmuls need is restored by two
    SBUF->SBUF partition-shift DMAs per region.
  - K/V projection uses TWO stationaries: [Wk|Wv] for even-slot chunks
    and [Wv|Wk] for odd-slot chunks, so K of chunk 2m+1 lands on
    partitions 64..127 directly. k2_sb packs [K(2m); K(2m+1)] per pair.
    V^T -> V by PE transpose as before (V sits in rows 64:128 / 0:64 of
    the even/odd transposed chunks respectively).

P^T = exp(S^T/8) is split across engines:
  - ACT path (diagonal + half the interior pairs): exp on the scalar
    engine from f32 PSUM (diagonal pair multiplied by the input mask).
  - DVE path (other interior pairs): Schraudolph bit-trick exp on the
    vector engine -- one fused tensor_scalar computes
    int16(S * (2^10/ln2)/8 + B); those int16 bits reinterpreted as
    float16 ARE approximately exp(S/8) (sawtooth rel-err ~2%, zero-mean
    after softmax normalization; interior pairs only feed rows with
    >=512 keys so the noise averages out -- measured no change in final
    rel-err). The int16 tile is bitcast to float16 for the AV matmul
    (fp16 rather than f32r because the BIR verifier requires f32r
    operands to come from an f32r-rounding producer).

AV: O'^T(65,512) += [V|1].T @ P^T (ones column -> softmax denominators
ride along row 64). The partial [O'^T; l] (65, 4096) goes to DRAM; the
host adds the two parity partials, divides by the summed denominators,
un-permutes and transposes. No max-subtraction: logits are O(1).

Engine balance: exp split ACT/DVE (24/12 pairs); Q evacuations and fp16
V copies on the otherwise-idle scalar engine; masks/ost/compaction on
DVE. po/ps double-buffered; PE-transpose scratch shares the projection
PSUM slots to stay within 8 banks.
"""

import numpy as np

B, T, E, H = 4, 4096, 1024, 64
P = 128           # partitions
QC = 512          # query chunk (matmul moving free dim)
KC = 128          # key chunk
ETILES = E // P   # 8 contraction tiles
NKCH = T // KC // 2   # 16 local (parity) key chunks per core
NPAIR = NKCH // 2     # 8 local key-chunk pairs
NREG = 4          # 1024-column load/projection regions
NQCH = T // QC    # 8 query chunks
TLOC = T // 2     # 2048 local (own-parity) tokens

_CACHE = {}

# Schraudolph exp in fp16 space: bitcast_f16(int16(x * 2^10/ln2 + (15-C)*2^10))
# ~ exp(x). t stays in [11k, 20k] -- inside int16/fp16 normals for |logit|<=4.
_SCH_A16 = (2.0 ** 10) / float(np.log(2.0)) / 8.0   # folds in the 1/sqrt(64)
_SCH_B16 = (15.0 - 0.0436775) * (2.0 ** 10)


def _is_dve_pair(m: int, j: int) -> bool:
    """Interior pairs routed to the vector-engine Schraudolph exp."""
    return m < j and (m % 2 == 1)


def _build_graph():
    import concourse.bass as bass
    import concourse.tile as tile
    from concourse import bacc, mybir
    f32 = mybir.dt.float32
    i16 = mybir.dt.int16
    fp16 = mybir.dt.float16
    bf16 = mybir.dt.bfloat16
    AF = mybir.ActivationFunctionType
    ALU = mybir.AluOpType
    RC = T // NREG  # 1024 columns per region

    nc = bacc.Bacc("TRN2", target_bir_lowering=False, debug=False, num_devices=8)
    xTa_d = nc.dram_tensor("xTa", [E, T], bf16, kind="ExternalInput").ap()
    wkv_d = nc.dram_tensor("wkv", [E, P], bf16, kind="ExternalInput").ap()
    wvk_d = nc.dram_tensor("wvk", [E, P], bf16, kind="ExternalInput").ap()
    wq_d = nc.dram_tensor("wq", [E, H], bf16, kind="ExternalInput").ap()
    dmask_d = nc.dram_tensor("dmask", [P, 2, QC], bf16, kind="ExternalInput").ap()
    ident_d = nc.dram_tensor("ident", [P, P], bf16, kind="ExternalInput").ap()
    out_d = nc.dram_tensor("o", [H + 1, NQCH, QC], f32, kind="ExternalOutput").ap()

    with tile.TileContext(nc) as tc:
        with (
            tc.tile_pool(name="consts", bufs=1) as consts,
            tc.tile_pool(name="xin", bufs=4) as xin,
            tc.tile_pool(name="big", bufs=1) as big,
            tc.tile_pool(name="work", bufs=3) as work,
            tc.tile_pool(name="psum", bufs=1, space="PSUM") as psum,
        ):
            # ---- constants ----
            wq_sb = consts.tile([P, ETILES, H], bf16)
            ident = consts.tile([P, P], bf16)
            wkv_sb = consts.tile([P, ETILES, P], bf16)
            wvk_sb = consts.tile([P, ETILES, P], bf16)
            dmask_sb = consts.tile([P, 2, QC], bf16)
            ones32 = consts.tile([P, 1], f32)
            nc.vector.memset(ones32[:], 1.0)

            # ---- projections ----
            # Per-REGION result tiles (not one monolithic tile) so the
            # attention for query chunk j only depends on regions <= j//2 --
            # Tile then overlaps late projections with early attention.
            k2 = []     # [K(2m); K(2m+1)] per pair, 2 pairs per region
            q2 = []     # [Q; Q] per region
            vs = []     # V chunks (+ones col), 4 per region
            vsh = []    # fp16 twin for the DVE-path AV
            for r in range(NREG):
                k2.append(big.tile([P, 2, KC], bf16, tag=f"k2_{r}", name=f"k2_{r}"))
                q2.append(big.tile([P, RC], bf16, tag=f"q2_{r}", name=f"q2_{r}"))
                vs.append(big.tile([P, 4, H + 1], bf16, tag=f"vs_{r}", name=f"vs_{r}"))
                nc.vector.tensor_copy(vs[r][:, :, H:H + 1],
                                      ones32[:, None, :].to_broadcast((P, 4, 1)))
                vsh.append(big.tile([P, 4, H + 1], fp16, tag=f"vsh_{r}", name=f"vsh_{r}"))
                nc.vector.memset(vsh[r][:, :, H:H + 1], 1.0)

            for r in range(NREG):
                xt = xin.tile([P, ETILES, RC], bf16, tag="xt")
                if r == 0:
                    # first x chunk, then the weights the first matmuls
                    # need, then the bulk -- extra dma_starts ahead of the
                    # x stream delay it ~0.5us each
                    nc.sync.dma_start(xt[:, 0], xTa_d[0:P, 0:RC])
                    nc.sync.dma_start(
                        wq_sb[:], wq_d.rearrange("(ko p) m -> p ko m", p=P))
                    for ko in range(1, ETILES):
                        nc.sync.dma_start(
                            xt[:, ko], xTa_d[ko * P:(ko + 1) * P, 0:RC])
                    nc.sync.dma_start(
                        wkv_sb[:], wkv_d.rearrange("(ko p) m -> p ko m", p=P))
                    nc.sync.dma_start(
                        wvk_sb[:], wvk_d.rearrange("(ko p) m -> p ko m", p=P))
                    nc.sync.dma_start(ident[:], ident_d[:])
                    nc.sync.dma_start(dmask_sb[:], dmask_d[:])
                else:
                    for ko in range(ETILES):
                        nc.sync.dma_start(
                            xt[:, ko],
                            xTa_d[ko * P:(ko + 1) * P, r * RC:(r + 1) * RC])
                # Q for both 512-chunks at once: col-tiled concurrent M=64
                # matmul groups (separate banks; the two groups' matmuls
                # interleave in the PE stream and run on disjoint column
                # strips of the array)
                pq_a = psum.tile([P, QC], f32, tag="proj", bufs=2,
                                 name=f"pq_a_{r}")
                pq_b = psum.tile([P, QC], f32, tag="proj", bufs=2,
                                 name=f"pq_b_{r}")
                for ko in range(ETILES):
                    nc.tensor.matmul(pq_a[0:H, :], wq_sb[:, ko],
                                     xt[:, ko, 0:QC],
                                     start=(ko == 0), stop=(ko == ETILES - 1))
                    nc.tensor.matmul(pq_b[H:P, :], wq_sb[:, ko],
                                     xt[:, ko, QC:RC],
                                     start=(ko == 0), stop=(ko == ETILES - 1))
                nc.scalar.copy(q2[r][0:H, 0:QC], pq_a[0:H, :])
                nc.scalar.copy(q2[r][H:P, QC:RC], pq_b[H:P, :])
                # restore the [Q;Q] row duplication for the row-tiled S
                nc.sync.dma_start(q2[r][H:P, 0:QC], q2[r][0:H, 0:QC])
                nc.sync.dma_start(q2[r][0:H, QC:RC], q2[r][H:P, QC:RC])
                # K,V for the region's 4 own-parity chunks. Region blocks sit
                # at 128-block positions {0,2,4,6}; local chunks 4r..4r+3 map
                # to positions {0,2,4,6} in order, so even-slot chunks
                # (4r,4r+2) are at w=0 and odd-slot (4r+1,4r+3) at w=2 of the
                # (a w c) split below. The PE crashes on strided moving
                # operands, so compact on DVE first.
                xr = xt.rearrange("p ko (a w c) -> p ko a w c", w=4, c=KC)
                xkv_e = work.tile([P, ETILES, 2, KC], bf16, tag="xkv_e", bufs=2)
                xkv_o = work.tile([P, ETILES, 2, KC], bf16, tag="xkv_o", bufs=2)
                for ko in range(ETILES):
                    nc.vector.tensor_copy(xkv_e[:, ko], xr[:, ko, :, 0])
                    nc.vector.tensor_copy(xkv_o[:, ko], xr[:, ko, :, 2])
                pkv_e = psum.tile([P, 2, KC], f32, tag="proj", bufs=2,
                                  name=f"pkv_e_{r}")
                pkv_o = psum.tile([P, 2, KC], f32, tag="proj", bufs=2,
                                  name=f"pkv_o_{r}")
                for ko in range(ETILES):
                    nc.tensor.matmul(pkv_e[:], wkv_sb[:, ko], xkv_e[:, ko],
                                     start=(ko == 0), stop=(ko == ETILES - 1))
                for ko in range(ETILES):
                    nc.tensor.matmul(pkv_o[:], wvk_sb[:, ko], xkv_o[:, ko],
                                     start=(ko == 0), stop=(ko == ETILES - 1))
                # kvt_e rows = [K;V] of chunks (4r, 4r+2); kvt_o = [V;K] of
                # (4r+1, 4r+3). K rows feed k2_sb pairs 2r/2r+1 directly.
                kvt_e = work.tile([P, 2, KC], bf16, tag="kvt_e", bufs=2)
                kvt_o = work.tile([P, 2, KC], bf16, tag="kvt_o", bufs=2)
                nc.vector.tensor_copy(kvt_e[:], pkv_e[:])
                nc.vector.tensor_copy(kvt_o[:], pkv_o[:])
                nc.vector.tensor_copy(k2[r][0:H], kvt_e[0:H])
                nc.vector.tensor_copy(k2[r][H:P], kvt_o[H:P])
                # V^T -> V by PE transpose (scratch shares "proj" PSUM slots)
                for s in range(4):
                    i = 4 * r + s            # local chunk index
                    kt = (kvt_e, kvt_o)[s % 2]
                    vcols = (slice(H, P), slice(0, H))[s % 2]
                    ptr = psum.tile([P, P], bf16, tag="proj", bufs=2,
                                    name=f"ptr_{i}")
                    nc.tensor.transpose(ptr[:], kt[:, s // 2], ident[:])
                    nc.vector.tensor_copy(vs[r][:, s, 0:H], ptr[:, vcols])
                    nc.scalar.copy(vsh[r][:, s, 0:H], ptr[:, vcols])

            # ---- attention (partial, own-parity keys) ----
            # j=1 then j=0 first (both only need region 0, filling PE idle
            # during later projections), then j ascending with region arrival
            for j in [1, 0] + list(range(2, NQCH)):
                npair = j + 1  # local kchunk pairs; extent = 2j+2 chunks
                po = psum.tile([H + 1, QC], f32, tag="po", bufs=2, name=f"po_{j}")
                qr, qh = q2[j // 2], (j % 2) * QC
                qlo = qr[0:H, qh:qh + QC]
                qhi = qr[H:P, qh:qh + QC]

                def s_pair(m):
                    # two concurrent 64-contraction matmuls on disjoint PE
                    # row strips (tile_position auto-derives from the 0/64
                    # operand base partitions)
                    ps = psum.tile([P, 2, QC], f32, tag="ps", bufs=2,
                                   name=f"ps_{j}_{m}")
                    nc.tensor.matmul(ps[:, 0], k2[m // 2][0:H, m % 2], qlo,
                                     start=True, stop=True)
                    nc.tensor.matmul(ps[:, 1], k2[m // 2][H:P, m % 2], qhi,
                                     start=True, stop=True)
                    return ps

                def exp_pair(m, ps):
                    if _is_dve_pair(m, j):
                        pts = work.tile([P, 2, QC], i16, tag="pts", bufs=4,
                                        name=f"pts_{j}_{m}")
                        nc.vector.tensor_scalar(pts[:], ps[:], _SCH_A16,
                                                _SCH_B16, ALU.mult, ALU.add)
                        return ("sch", pts)
                    pt = work.tile([P, 2, QC], bf16, tag="pt", bufs=6,
                                   name=f"pt_{j}_{m}")
                    nc.scalar.activation(pt[:], ps[:], AF.Exp,
                                         scale=float(H) ** -0.5)
                    if m == j:  # diagonal pair
                        nc.vector.tensor_tensor(pt[:], pt[:], dmask_sb[:],
                                                ALU.mult)
                    return ("act", pt)

                def av_pair(m, kt, first, last):
                    kind, t = kt
                    for u in range(2):
                        i = 2 * m + u
                        if kind == "sch":
                            nc.tensor.matmul(po[:], vsh[i // 4][:, i % 4, :],
                                             t[:, u].bitcast(fp16),
                                             start=(first and u == 0),
                                             stop=(last and u == 1))
                        else:
                            nc.tensor.matmul(po[:], vs[i // 4][:, i % 4, :],
                                             t[:, u],
                                             start=(first and u == 0),
                                             stop=(last and u == 1))

                # diagonal (masked) pair first so the DVE mask never gates the
                # final AV; software-pipelined emission: S(next) before AV(cur)
                order = [j] + list(range(j))
                ps = s_pair(order[0])
                pt = exp_pair(order[0], ps)
                for idx in range(1, npair):
                    ps2 = s_pair(order[idx])
                    av_pair(order[idx - 1], pt, idx - 1 == 0, False)
                    pt = exp_pair(order[idx], ps2)
                av_pair(order[-1], pt, npair == 1, True)

                ost = work.tile([H + 1, QC], f32, tag="ost", bufs=2)
                nc.vector.tensor_copy(ost[:], po[:])
                nc.sync.dma_start(out_d[:, j], ost[:])

    nc.compile()
    return nc


def _get_graph():
    if "g" not in _CACHE:
        _CACHE["g"] = _build_graph()
    return _CACHE["g"]


def _perm(p: int) -> np.ndarray:
    """Token column permutation for parity p: own-parity 128-blocks at even
    block positions (identity for p=0, adjacent-block swap for p=1)."""
    blocks = np.arange(T // KC).reshape(-1, 2)
    if p == 1:
        blocks = blocks[:, ::-1]
    return (blocks.reshape(-1)[:, None] * KC + np.arange(KC)[None, :]).reshape(-1)


def _make_masks(p: int) -> np.ndarray:
    """Diagonal-pair masks in permuted column space: column t' of a query
    chunk is global token offset sigma(t'); diag chunks have global key
    offsets 128*p (slot 0) and 128*(p+2) (slot 1) within the chunk."""
    perm = _perm(p)
    sigma = perm[:QC] % QC  # within-chunk token offset pattern (j-independent)
    s = np.arange(P)[:, None]
    m = np.empty((P, 2, QC), np.float32)
    m[:, 0] = (sigma[None, :] - s - KC * p) >= 0
    m[:, 1] = (sigma[None, :] - s - KC * (p + 2)) >= 0
    return m


def _run(x, Wk, Wq, Wv, trace=False):
    from concourse.bass_utils import run_bass_kernel_spmd
    import ml_dtypes

    x = np.asarray(x, dtype=np.float32)
    Wk = np.asarray(Wk, dtype=np.float32)
    Wq = np.asarray(Wq, dtype=np.float32)
    Wv = np.asarray(Wv, dtype=np.float32)

    conv = lambda a: np.asarray(a, dtype=ml_dtypes.bfloat16)
    wkv = conv(np.concatenate([Wk.T, Wv.T], axis=1))
    wvk = conv(np.concatenate([Wv.T, Wk.T], axis=1))
    wq = conv(Wq.T)
    masks = [conv(_make_masks(0)), conv(_make_masks(1))]
    ident_np = conv(np.eye(P, dtype=np.float32))
    perms = [_perm(0), _perm(1)]

    in_maps = []
    xTb = {}
    for c in range(8):
        b, p = c // 2, c % 2
        if (b, p) not in xTb:
            xTb[(b, p)] = conv(x[b].T[:, perms[p]])
        in_maps.append({"xTa": xTb[(b, p)], "wkv": wkv, "wvk": wvk,
                        "wq": wq, "dmask": masks[p], "ident": ident_np})

    nc = _get_graph()
    res = run_bass_kernel_spmd(nc, in_maps, core_ids=list(range(8)), trace=trace)

    out = np.empty((B, T, H), dtype=np.float32)
    for b in range(B):
        o0 = res.results[2 * b]["o"].reshape(H + 1, T)
        o1 = res.results[2 * b + 1]["o"].reshape(H + 1, T)
        # p=1 columns are block-swapped; un-permute before merging
        o1 = o1[:, perms[1]]
        s = o0 + o1
        out[b] = (s[0:H] / s[H:H + 1]).T
    return out, res.exec_time_ns


def kernel(x, Wk, Wq, Wv):
    out, _ = _run(x, Wk, Wq, Wv)
    return out


# revision 12
# speedup vs baseline: 1.1973x; 1.1973x over previous
"""Distributed Trainium2 kernel for a single causal attention head.

Module: k,q,v = x@W{k,q,v}.T ; a = softmax(causal(q@k.T/sqrt(64))) ; out = a@v
Shapes: x (4, 4096, 1024) f32; W* (64, 1024) f32; out (4, 4096, 64) f32.

Sharding (one SPMD launch, 8 cores, no collectives): 4 batches x 2
key-parity halves. Core c: batch b=c//2, parity p=c%2. The 32 key chunks
(128 tokens) of a batch are split by parity (even chunks -> p=0, odd ->
p=1): for query chunk j (512 tokens), each core processes exactly 2j+2
of its local key chunks; its two diagonal mask tiles arrive as input
data. The host hands each core x[b].T with token columns permuted so the
core's own-parity key blocks sit at even 128-block positions.

Row-tiled S matmuls: the S contraction is only 64-deep (head dim), so
each local key-chunk PAIR runs as TWO concurrent matmuls on disjoint
64-row halves of the PE array (tile_position row strips) -- the pair
costs ~512 cycles instead of 1024. To feed this:
  - Q projection uses stationary [Wq|Wq] so the PSUM result is [Q;Q]
    (the moving-cycle cost is identical to [Wq|0]; the duplicate rows
    64..127 are exactly what the row-strip-1 matmuls must stream).
  - K/V projection uses TWO stationaries: [Wk|Wv] for even-slot chunks
    and [Wv|Wk] for odd-slot chunks, so K of chunk 2m+1 lands on
    partitions 64..127 directly. k2_sb packs [K(2m); K(2m+1)] per pair.
    V^T -> V by PE transpose as before (V sits in rows 64:128 / 0:64 of
    the even/odd transposed chunks respectively).

P^T = exp(S^T/8) is split across engines:
  - ACT path (diagonal + half the interior pairs): exp on the scalar
    engine from f32 PSUM (diagonal pair multiplied by the input mask).
  - DVE path (other interior pairs): Schraudolph bit-trick exp on the
    vector engine -- one fused tensor_scalar computes
    int16(S * (2^10/ln2)/8 + B); those int16 bits reinterpreted as
    float16 ARE approximately exp(S/8) (sawtooth rel-err ~2%, zero-mean
    after softmax normalization; interior pairs only feed rows with
    >=512 keys so the noise averages out -- measured no change in final
    rel-err). The int16 tile is bitcast to float16 for the AV matmul
    (fp16 rather than f32r because the BIR verifier requires f32r
    operands to come from an f32r-rounding producer).

AV: O'^T(65,512) += [V|1].T @ P^T (ones column -> softmax denominators
ride along row 64). The partial [O'^T; l] (65, 4096) goes to DRAM; the
host adds the two parity partials, divides by the summed denominators,
un-permutes and transposes. No max-subtraction: logits are O(1).

Engine balance: exp split ACT/DVE (24/12 pairs); Q evacuations and fp16
V copies on the otherwise-idle scalar engine; masks/ost/compaction on
DVE. po/ps double-buffered; PE-transpose scratch shares the projection
PSUM slots to stay within 8 banks.
"""

import numpy as np

B, T, E, H = 4, 4096, 1024, 64
P = 128           # partitions
QC = 512          # query chunk (matmul moving free dim)
KC = 128          # key chunk
ETILES = E // P   # 8 contraction tiles
NKCH = T // KC // 2   # 16 local (parity) key chunks per core
NPAIR = NKCH // 2     # 8 local key-chunk pairs
NREG = 4          # 1024-column load/projection regions
NQCH = T // QC    # 8 query chunks
TLOC = T // 2     # 2048 local (own-parity) tokens

_CACHE = {}

# Schraudolph exp in fp16 space: bitcast_f16(int16(x * 2^10/ln2 + (15-C)*2^10))
# ~ exp(x). t stays in [11k, 20k] -- inside int16/fp16 normals for |logit|<=4.
_SCH_A16 = (2.0 ** 10) / float(np.log(2.0)) / 8.0   # folds in the 1/sqrt(64)
_SCH_B16 = (15.0 - 0.0436775) * (2.0 ** 10)


def _is_dve_pair(m: int, j: int) -> bool:
    """Interior pairs routed to the vector-engine Schraudolph exp."""
    return m < j and (m % 2 == 1)


def _build_graph():
    import concourse.bass as bass
    import concourse.tile as tile
    from concourse import bacc, mybir
    f32 = mybir.dt.float32
    i16 = mybir.dt.int16
    fp16 = mybir.dt.float16
    bf16 = mybir.dt.bfloat16
    AF = mybir.ActivationFunctionType
    ALU = mybir.AluOpType
    RC = T // NREG  # 1024 columns per region

    nc = bacc.Bacc("TRN2", target_bir_lowering=False, debug=False, num_devices=8)
    xTa_d = nc.dram_tensor("xTa", [E, T], bf16, kind="ExternalInput").ap()
    wkv_d = nc.dram_tensor("wkv", [E, P], bf16, kind="ExternalInput").ap()
    wvk_d = nc.dram_tensor("wvk", [E, P], bf16, kind="ExternalInput").ap()
    wqq_d = nc.dram_tensor("wqq", [E, P], bf16, kind="ExternalInput").ap()
    dmask_d = nc.dram_tensor("dmask", [P, 2, QC], bf16, kind="ExternalInput").ap()
    ident_d = nc.dram_tensor("ident", [P, P], bf16, kind="ExternalInput").ap()
    out_d = nc.dram_tensor("o", [H + 1, NQCH, QC], f32, kind="ExternalOutput").ap()

    with tile.TileContext(nc) as tc:
        with (
            tc.tile_pool(name="consts", bufs=1) as consts,
            tc.tile_pool(name="xin", bufs=4) as xin,
            tc.tile_pool(name="big", bufs=1) as big,
            tc.tile_pool(name="work", bufs=3) as work,
            tc.tile_pool(name="psum", bufs=1, space="PSUM") as psum,
        ):
            # ---- constants ----
            wqq_sb = consts.tile([P, ETILES, P], bf16)
            nc.sync.dma_start(wqq_sb[:], wqq_d.rearrange("(ko p) m -> p ko m", p=P))
            ident = consts.tile([P, P], bf16)
            wkv_sb = consts.tile([P, ETILES, P], bf16)
            wvk_sb = consts.tile([P, ETILES, P], bf16)
            dmask_sb = consts.tile([P, 2, QC], bf16)
            ones32 = consts.tile([P, 1], f32)
            nc.vector.memset(ones32[:], 1.0)

            # ---- projections ----
            # Per-REGION result tiles (not one monolithic tile) so the
            # attention for query chunk j only depends on regions <= j//2 --
            # Tile then overlaps late projections with early attention.
            k2 = []     # [K(2m); K(2m+1)] per pair, 2 pairs per region
            q2 = []     # [Q; Q] per region
            vs = []     # V chunks (+ones col), 4 per region
            vsh = []    # fp16 twin for the DVE-path AV
            for r in range(NREG):
                k2.append(big.tile([P, 2, KC], bf16, tag=f"k2_{r}", name=f"k2_{r}"))
                q2.append(big.tile([P, RC], bf16, tag=f"q2_{r}", name=f"q2_{r}"))
                vs.append(big.tile([P, 4, H + 1], bf16, tag=f"vs_{r}", name=f"vs_{r}"))
                nc.vector.tensor_copy(vs[r][:, :, H:H + 1],
                                      ones32[:, None, :].to_broadcast((P, 4, 1)))
                vsh.append(big.tile([P, 4, H + 1], fp16, tag=f"vsh_{r}", name=f"vsh_{r}"))
                nc.vector.memset(vsh[r][:, :, H:H + 1], 1.0)

            for r in range(NREG):
                xt = xin.tile([P, ETILES, RC], bf16, tag="xt")
                for ko in range(ETILES):
                    nc.sync.dma_start(
                        xt[:, ko],
                        xTa_d[ko * P:(ko + 1) * P, r * RC:(r + 1) * RC])
                if r == 0:
                    nc.sync.dma_start(ident[:], ident_d[:])
                    nc.sync.dma_start(
                        wkv_sb[:], wkv_d.rearrange("(ko p) m -> p ko m", p=P))
                    nc.sync.dma_start(
                        wvk_sb[:], wvk_d.rearrange("(ko p) m -> p ko m", p=P))
                    nc.sync.dma_start(dmask_sb[:], dmask_d[:])
                # [Q; Q] for both 512-chunks of the region
                for half in range(2):
                    pq = psum.tile([P, QC], f32, tag="proj", bufs=2)
                    for ko in range(ETILES):
                        nc.tensor.matmul(pq[:], wqq_sb[:, ko],
                                         xt[:, ko, half * QC:(half + 1) * QC],
                                         start=(ko == 0), stop=(ko == ETILES - 1))
                    nc.scalar.copy(q2[r][:, half * QC:(half + 1) * QC], pq[:])
                # K,V for the region's 4 own-parity chunks. Region blocks sit
                # at 128-block positions {0,2,4,6}; local chunks 4r..4r+3 map
                # to positions {0,2,4,6} in order, so even-slot chunks
                # (4r,4r+2) are at w=0 and odd-slot (4r+1,4r+3) at w=2 of the
                # (a w c) split below. The PE crashes on strided moving
                # operands, so compact on DVE first.
                xr = xt.rearrange("p ko (a w c) -> p ko a w c", w=4, c=KC)
                xkv_e = work.tile([P, ETILES, 2, KC], bf16, tag="xkv_e", bufs=2)
                xkv_o = work.tile([P, ETILES, 2, KC], bf16, tag="xkv_o", bufs=2)
                for ko in range(ETILES):
                    nc.vector.tensor_copy(xkv_e[:, ko], xr[:, ko, :, 0])
                    nc.vector.tensor_copy(xkv_o[:, ko], xr[:, ko, :, 2])
                pkv_e = psum.tile([P, 2, KC], f32, tag="proj", bufs=2,
                                  name=f"pkv_e_{r}")
                pkv_o = psum.tile([P, 2, KC], f32, tag="proj", bufs=2,
                                  name=f"pkv_o_{r}")
                for ko in range(ETILES):
                    nc.tensor.matmul(pkv_e[:], wkv_sb[:, ko], xkv_e[:, ko],
                                     start=(ko == 0), stop=(ko == ETILES - 1))
                for ko in range(ETILES):
                    nc.tensor.matmul(pkv_o[:], wvk_sb[:, ko], xkv_o[:, ko],
                                     start=(ko == 0), stop=(ko == ETILES - 1))
                # kvt_e rows = [K;V] of chunks (4r, 4r+2); kvt_o = [V;K] of
                # (4r+1, 4r+3). K rows feed k2_sb pairs 2r/2r+1 directly.
                kvt_e = work.tile([P, 2, KC], bf16, tag="kvt_e", bufs=2)
                kvt_o = work.tile([P, 2, KC], bf16, tag="kvt_o", bufs=2)
                nc.vector.tensor_copy(kvt_e[:], pkv_e[:])
                nc.vector.tensor_copy(kvt_o[:], pkv_o[:])
                nc.vector.tensor_copy(k2[r][0:H], kvt_e[0:H])
                nc.vector.tensor_copy(k2[r][H:P], kvt_o[H:P])
                # V^T -> V by PE transpose (scratch shares "proj" PSUM slots)
                for s in range(4):
                    i = 4 * r + s            # local chunk index
                    kt = (kvt_e, kvt_o)[s % 2]
                    vcols = (slice(H, P), slice(0, H))[s % 2]
                    ptr = psum.tile([P, P], bf16, tag="proj", bufs=2,
                                    name=f"ptr_{i}")
                    nc.tensor.transpose(ptr[:], kt[:, s // 2], ident[:])
                    nc.vector.tensor_copy(vs[r][:, s, 0:H], ptr[:, vcols])
                    nc.scalar.copy(vsh[r][:, s, 0:H], ptr[:, vcols])

            # ---- attention (partial, own-parity keys) ----
            # j=1 then j=0 first (both only need region 0, filling PE idle
            # during later projections), then j ascending with region arrival
            for j in [1, 0] + list(range(2, NQCH)):
                npair = j + 1  # local kchunk pairs; extent = 2j+2 chunks
                po = psum.tile([H + 1, QC], f32, tag="po", bufs=2, name=f"po_{j}")
                qr, qh = q2[j // 2], (j % 2) * QC
                qlo = qr[0:H, qh:qh + QC]
                qhi = qr[H:P, qh:qh + QC]

                def s_pair(m):
                    # two concurrent 64-contraction matmuls on disjoint PE
                    # row strips (tile_position auto-derives from the 0/64
                    # operand base partitions)
                    ps = psum.tile([P, 2, QC], f32, tag="ps", bufs=2,
                                   name=f"ps_{j}_{m}")
                    nc.tensor.matmul(ps[:, 0], k2[m // 2][0:H, m % 2], qlo,
                                     start=True, stop=True)
                    nc.tensor.matmul(ps[:, 1], k2[m // 2][H:P, m % 2], qhi,
                                     start=True, stop=True)
                    return ps

                def exp_pair(m, ps):
                    if _is_dve_pair(m, j):
                        pts = work.tile([P, 2, QC], i16, tag="pts", bufs=4,
                                        name=f"pts_{j}_{m}")
                        nc.vector.tensor_scalar(pts[:], ps[:], _SCH_A16,
                                                _SCH_B16, ALU.mult, ALU.add)
                        return ("sch", pts)
                    pt = work.tile([P, 2, QC], bf16, tag="pt", bufs=6,
                                   name=f"pt_{j}_{m}")
                    nc.scalar.activation(pt[:], ps[:], AF.Exp,
                                         scale=float(H) ** -0.5)
                    if m == j:  # diagonal pair
                        nc.vector.tensor_tensor(pt[:], pt[:], dmask_sb[:],
                                                ALU.mult)
                    return ("act", pt)

                def av_pair(m, kt, first, last):
                    kind, t = kt
                    for u in range(2):
                        i = 2 * m + u
                        if kind == "sch":
                            nc.tensor.matmul(po[:], vsh[i // 4][:, i % 4, :],
                                             t[:, u].bitcast(fp16),
                                             start=(first and u == 0),
                                             stop=(last and u == 1))
                        else:
                            nc.tensor.matmul(po[:], vs[i // 4][:, i % 4, :],
                                             t[:, u],
                                             start=(first and u == 0),
                                             stop=(last and u == 1))

                # diagonal (masked) pair first so the DVE mask never gates the
                # final AV; software-pipelined emission: S(next) before AV(cur)
                order = [j] + list(range(j))
                ps = s_pair(order[0])
                pt = exp_pair(order[0], ps)
                for idx in range(1, npair):
                    ps2 = s_pair(order[idx])
                    av_pair(order[idx - 1], pt, idx - 1 == 0, False)
                    pt = exp_pair(order[idx], ps2)
                av_pair(order[-1], pt, npair == 1, True)

                ost = work.tile([H + 1, QC], f32, tag="ost", bufs=2)
                nc.vector.tensor_copy(ost[:], po[:])
                nc.sync.dma_start(out_d[:, j], ost[:])

    nc.compile()
    return nc


def _get_graph():
    if "g" not in _CACHE:
        _CACHE["g"] = _build_graph()
    return _CACHE["g"]


def _perm(p: int) -> np.ndarray:
    """Token column permutation for parity p: own-parity 128-blocks at even
    block positions (identity for p=0, adjacent-block swap for p=1)."""
    blocks = np.arange(T // KC).reshape(-1, 2)
    if p == 1:
        blocks = blocks[:, ::-1]
    return (blocks.reshape(-1)[:, None] * KC + np.arange(KC)[None, :]).reshape(-1)


def _make_masks(p: int) -> np.ndarray:
    """Diagonal-pair masks in permuted column space: column t' of a query
    chunk is global token offset sigma(t'); diag chunks have global key
    offsets 128*p (slot 0) and 128*(p+2) (slot 1) within the chunk."""
    perm = _perm(p)
    sigma = perm[:QC] % QC  # within-chunk token offset pattern (j-independent)
    s = np.arange(P)[:, None]
    m = np.empty((P, 2, QC), np.float32)
    m[:, 0] = (sigma[None, :] - s - KC * p) >= 0
    m[:, 1] = (sigma[None, :] - s - KC * (p + 2)) >= 0
    return m


def _run(x, Wk, Wq, Wv, trace=False):
    from concourse.bass_utils import run_bass_kernel_spmd
    import ml_dtypes

    x = np.asarray(x, dtype=np.float32)
    Wk = np.asarray(Wk, dtype=np.float32)
    Wq = np.asarray(Wq, dtype=np.float32)
    Wv = np.asarray(Wv, dtype=np.float32)

    conv = lambda a: np.asarray(a, dtype=ml_dtypes.bfloat16)
    wkv = conv(np.concatenate([Wk.T, Wv.T], axis=1))
    wvk = conv(np.concatenate([Wv.T, Wk.T], axis=1))
    wqq = conv(np.concatenate([Wq.T, Wq.T], axis=1))
    masks = [conv(_make_masks(0)), conv(_make_masks(1))]
    ident_np = conv(np.eye(P, dtype=np.float32))
    perms = [_perm(0), _perm(1)]

    in_maps = []
    xTb = {}
    for c in range(8):
        b, p = c // 2, c % 2
        if (b, p) not in xTb:
            xTb[(b, p)] = conv(x[b].T[:, perms[p]])
        in_maps.append({"xTa": xTb[(b, p)], "wkv": wkv, "wvk": wvk,
                        "wqq": wqq, "dmask": masks[p], "ident": ident_np})

    nc = _get_graph()
    res = run_bass_kernel_spmd(nc, in_maps, core_ids=list(range(8)), trace=trace)

    out = np.empty((B, T, H), dtype=np.float32)
    for b in range(B):
        o0 = res.results[2 * b]["o"].reshape(H + 1, T)
        o1 = res.results[2 * b + 1]["o"].reshape(H + 1, T)
        # p=1 columns are block-swapped; un-permute before merging
        o1 = o1[:, perms[1]]
        s = o0 + o1
        out[b] = (s[0:H] / s[H:H + 1]).T
    return out, res.exec_time_ns


def kernel(x, Wk, Wq, Wv):
    out, _ = _run(x, Wk, Wq, Wv)
    return out
